# revision 21
# baseline (speedup 1.0000x reference)
"""Trainium2 Bass kernel for nn_Decoder_75505525064316 (dense_mlp).

Reference computation (all biases are ZERO by construction in setup_inputs):
    y[n,d] = sum_l z[n,l] * |Wp[d,l]|                  # [N, 128]
    h1     = relu(y[...,None] * W1)                    # [N, 128, 32]
    h2     = relu(einsum('ndh,dkh->ndk', h1, W2))      # [N, 128, 32]
    x      = einsum('ndh,dh->nd', h2, W3)              # [N, 128]
    out    = |x|

Because each per-feature MLP takes a SCALAR input s = y[n,d] and every bias is
zero, each layer is positively homogeneous: f(a*s) = a*f(s) for a >= 0.  Hence
the entire per-feature MLP is piecewise-linear with a single breakpoint at 0:
    s >= 0:  x = s * gp_d,   gp_d = W3[d] @ relu(W2[d] @ relu( W1[d]))
    s <  0:  x = -s * gn_d,  gn_d = W3[d] @ relu(W2[d] @ relu(-W1[d]))
so  |x| = max(|gp_d| * s, -|gn_d| * s).

Device kernel (data-parallel over batch N across 8 cores; tokens sharded):
    per 512-token tile:
      y   = matmul(lhsT=|Wp|^T [64,128], rhs=z^T tile [64,512]) -> PSUM [128,512]
      ps  = Relu(cp * y)          (ScalarE activation, per-partition scale cp)
      out = max(cn * y, ps)       (VectorE fused scalar_tensor_tensor)
ps = max(cp*y, 0) so out = max(cp*y, cn*y, 0) = max(cp*y, cn*y) exactly
(cp = |gp| >= 0, cn = -|gn| <= 0).  The kernel is memory-bound: 2 MB in +
4 MB out per core.
"""

import numpy as np

import concourse.bacc as bacc
import concourse.mybir as mybir
import concourse.tile as tile
from concourse import bass_utils

N_CORES = 8
N_TOTAL = 65536
LATENT = 64
OUT = 128
N_PER_CORE = N_TOTAL // N_CORES  # 8192
T = 512                          # token tile (one PSUM bank of fp32)

_nc_cache = {}


def build_nc(repeats: int = 1, groups=(1, 1, 2, 4, 4, 4), io_bufs: int = 3,
             psum_bufs: int = 6, ps_bufs: int = 6, warmup: int = 2,
             out_on_scalar: bool = True, staggered: bool = False,
             f32r: bool = False, split: bool = False,
             out_eng: str = 'scalar', ct: int = 512,
             out_split: bool = False, z0_first: bool = False,
             out_chunk: int = 1):
    """Build + compile the per-core Bass program (SPMD: same NEFF, 8 cores).

    repeats > 1 wraps the whole computation in an on-device For_i loop (for
    wall-clock benchmarking with dispatch overhead amortized); the body is
    idempotent so results are unchanged.

    groups: compute tiles (of T tokens) per input dma_start — each dma_start
    costs ~650 ns of serialized issue on the issuing sequencer, so batching
    gets the DMA engines to line rate; small leading groups shorten the
    time-to-first-matmul ramp.
    warmup: dummy matmuls issued at kernel start to warm the PE HAM clock
    gate (cold PE runs at 1.2 GHz for the first ~3.4 us otherwise).
    out_on_scalar: issue output DMAs from the ACT sequencer's HWDGE queue so
    they don't serialize with input-DMA issue on SP.
    """
    key = (repeats, tuple(groups), io_bufs, psum_bufs, ps_bufs, warmup,
           out_on_scalar, staggered, f32r, split, out_eng, ct, out_split,
           z0_first, out_chunk)
    if key in _nc_cache:
        return _nc_cache[key]

    assert sum(groups) * T == N_PER_CORE

    nc = bacc.Bacc("TRN2", target_bir_lowering=False, debug=False)

    if split:
        mmdt = mybir.dt.bfloat16
        zdim, wcols = 2 * LATENT, 2 * OUT
    else:
        mmdt = mybir.dt.float32r if f32r else mybir.dt.float32
        zdim, wcols = LATENT, OUT
    zt = nc.dram_tensor("zt", [zdim, N_PER_CORE], mmdt,
                        kind="ExternalInput")
    wa = nc.dram_tensor("wa", [zdim, wcols], mmdt,
                        kind="ExternalInput")
    cc = nc.dram_tensor("cc", [OUT, 2], mybir.dt.float32, kind="ExternalInput")
    out = nc.dram_tensor("out", [OUT, N_PER_CORE], mybir.dt.float32,
                         kind="ExternalOutput")

    max_b = max(groups)

    with tile.TileContext(nc) as tc:
        with (
            tc.tile_pool(name="const", bufs=1) as cpool,
            tc.tile_pool(name="io", bufs=io_bufs) as io,
            tc.tile_pool(name="ps", bufs=ps_bufs) as pspool,
            tc.tile_pool(name="acc", bufs=psum_bufs, space="PSUM") as psum,
        ):
            pre = {}
            if z0_first:
                TB0 = T * groups[0]
                z0_sb = io.tile([zdim, T * max_b], mmdt, tag="z")
                nc.sync.dma_start(out=z0_sb[:, :TB0], in_=zt[:, 0:TB0])
                pre[0] = z0_sb
            w_sb = cpool.tile([zdim, wcols], mmdt)
            nc.sync.dma_start(out=w_sb, in_=wa[:, :])
            cc_sb = cpool.tile([OUT, 2], mybir.dt.float32)
            nc.sync.dma_start(out=cc_sb, in_=cc[:, :])
            cp_sb = cc_sb[:, 0:1]
            cn_sb = cc_sb[:, 1:2]

            if warmup:
                # Warm the PE HAM while the first z DMA is in flight: matmul
                # on the (already loaded or garbage) weight tile into a
                # scratch psum bank; consumed by a tiny DVE read so DCE
                # keeps it.
                wu_ps = psum.tile([OUT, OUT], mybir.dt.float32, tag="wu",
                                  bufs=1)
                wu_sb = cpool.tile([OUT, 1], mybir.dt.float32)
                for _ in range(warmup):
                    nc.tensor.matmul(wu_ps, lhsT=w_sb[:, :OUT],
                                     rhs=w_sb[:, :OUT],
                                     start=True, stop=True)
                nc.vector.tensor_copy(wu_sb, wu_ps[:, 0:1])

            if out_eng == 'alt':
                _engs = [nc.sync, nc.scalar]
                _cnt = [0]
                def dma_out(**kw):
                    _engs[_cnt[0] % 2].dma_start(**kw)
                    _cnt[0] += 1
            else:
                _eng = {'scalar': nc.scalar, 'sync': nc.sync,
                        'gpsimd': nc.gpsimd}[out_eng]
                def dma_out(**kw):
                    _eng.dma_start(**kw)

            def body():
                tok = 0
                for g, B in enumerate(groups):
                    TB = T * B
                    assert TB % ct == 0 or TB < ct
                    gsl = slice(tok, tok + TB)
                    if g in pre:
                        z_sb = pre.pop(g)
                    else:
                        z_sb = io.tile([zdim, T * max_b], mmdt, tag="z")
                        nc.sync.dma_start(out=z_sb[:, :TB], in_=zt[:, gsl])
                    o_sb = io.tile([OUT, T * max_b], mybir.dt.float32,
                                   tag="o")
                    for c0 in range(0, TB, ct):
                        cw = min(ct, TB - c0)
                        y_ps = psum.tile([OUT, ct], mybir.dt.float32, tag="y")
                        for j0 in range(0, cw, T):
                            jsl = slice(c0 + j0, c0 + j0 + T)
                            ysl = slice(j0, j0 + T)
                            if split:
                                nc.tensor.matmul(y_ps[:, ysl],
                                                 lhsT=w_sb[:, :OUT],
                                                 rhs=z_sb[:, jsl],
                                                 start=True, stop=False)
                                nc.tensor.matmul(y_ps[:, ysl],
                                                 lhsT=w_sb[:, OUT:],
                                                 rhs=z_sb[:, jsl],
                                                 start=False, stop=True)
                            else:
                                nc.tensor.matmul(y_ps[:, ysl], lhsT=w_sb,
                                                 rhs=z_sb[:, jsl],
                                                 start=True, stop=True)
                        ps_sb = pspool.tile([OUT, ct], mybir.dt.float32,
                                            tag="p")
                        nc.scalar.activation(
                            ps_sb[:, :cw], y_ps[:, :cw],
                            mybir.ActivationFunctionType.Relu, scale=cp_sb)
                        nc.vector.scalar_tensor_tensor(
                            o_sb[:, c0:c0 + cw], in0=y_ps[:, :cw],
                            scalar=cn_sb, in1=ps_sb[:, :cw],
                            op0=mybir.AluOpType.mult, op1=mybir.AluOpType.max)
                        if out_split:
                            oc = ct * out_chunk
                            c_end = c0 + cw
                            if c_end % oc == 0 or c_end == TB:
                                o0 = (c_end - 1) // oc * oc
                                dma_out(
                                    out=out[:, tok + o0:tok + c_end],
                                    in_=o_sb[:, o0:c_end])
                    if not out_split:
                        dma_out(out=out[:, gsl], in_=o_sb[:, :TB])
                    tok += TB

            if repeats == 1:
                body()
            else:
                with tc.For_i(0, repeats, 1, staggered_reset=staggered):
                    body()

    nc.compile()
    _nc_cache[key] = nc
    return nc


def make_in_maps(z, Wp, W1, b1, W2, b2, W3, b3, split=False):
    """Host-side precompute + shard. Returns per-core input dicts.

    split=True encodes z and the mixing weights as (bf16 hi, bf16 lo) pairs
    stacked along the contraction dim, so the device uses two full-rate
    K=128 bf16 matmuls instead of one quarter-rate K=64 fp32 matmul:
        y = [Whi;Whi]^T @ [zhi;zlo] + [Wlo;Wlo]^T @ [zhi;zlo]
          = (Whi+Wlo) @ (zhi+zlo)  ~=  W @ z  (split error ~2^-18)
    Same DMA byte count as fp32.
    """
    assert not np.any(b1) and not np.any(b2) and not np.any(b3), (
        "kernel assumes zero biases (guaranteed by setup_inputs); got nonzero")

    Wp64 = np.abs(Wp.astype(np.float64))
    W164 = W1.astype(np.float64)
    W264 = W2.astype(np.float64)
    W364 = W3.astype(np.float64)

    # gp[d] = W3[d] @ relu(W2[d] @ relu(W1[d])); gn with -W1.
    h1p = np.maximum(W164, 0.0)                     # [OUT, H1]
    h1n = np.maximum(-W164, 0.0)
    h2p = np.maximum(np.einsum('dkh,dh->dk', W264, h1p), 0.0)
    h2n = np.maximum(np.einsum('dkh,dh->dk', W264, h1n), 0.0)
    gp = np.einsum('dk,dk->d', W364, h2p)
    gn = np.einsum('dk,dk->d', W364, h2n)

    wa = np.ascontiguousarray(Wp64.T).astype(np.float32)          # [64, 128]
    cc = np.stack([np.abs(gp), -np.abs(gn)], axis=1).astype(np.float32)

    if split:
        from ml_dtypes import bfloat16
        w_hi = wa.astype(bfloat16)
        w_lo = (wa - w_hi.astype(np.float32)).astype(bfloat16)
        whh = np.concatenate([w_hi, w_hi], axis=0)                # [128, 128]
        wll = np.concatenate([w_lo, w_lo], axis=0)
        wa_dev = np.ascontiguousarray(
            np.concatenate([whh, wll], axis=1))                   # [128, 256]
    else:
        wa_dev = wa

    z = np.asarray(z, dtype=np.float32)
    in_maps = []
    for c in range(N_CORES):
        zc = z[c * N_PER_CORE:(c + 1) * N_PER_CORE, :]            # [8192, 64]
        zt = np.ascontiguousarray(zc.T)                           # [64, 8192]
        if split:
            from ml_dtypes import bfloat16
            z_hi = zt.astype(bfloat16)
            z_lo = (zt - z_hi.astype(np.float32)).astype(bfloat16)
            zt = np.ascontiguousarray(
                np.concatenate([z_hi, z_lo], axis=0))             # [128, 8192]
        in_maps.append({"zt": zt, "wa": wa_dev, "cc": cc})
    return in_maps


# Tuned on HW (see test.py): ~22.5 us/pass vs ~16.8 us pure-DMA floor.
BEST_CFG = dict(split=True, groups=(1, 1, 2, 4, 4, 4), warmup=4, io_bufs=4,
                psum_bufs=6, ps_bufs=6, out_eng='sync', out_split=True)


def kernel(z, Wp, W1, b1, W2, b2, W3, b3):
    nc = build_nc(**BEST_CFG)
    in_maps = make_in_maps(z, Wp, W1, b1, W2, b2, W3, b3,
                           split=BEST_CFG['split'])
    res = bass_utils.run_bass_kernel_spmd(nc, in_maps,
                                          core_ids=list(range(N_CORES)))
    outs = [res.results[c]["out"] for c in range(N_CORES)]        # [128, 8192]
    full = np.concatenate(outs, axis=1)                           # [128, 65536]
    return np.ascontiguousarray(full.T)                           # [65536, 128]
